# revision 15
# baseline (speedup 1.0000x reference)
"""Bass/Tile TRN2 kernel: multi-head attention with a local (sliding-window)
causal mask, window = 128, fp16 compute with fp32 PSUM accumulation.

Problem: x[2, 4096, 1024], 16 heads x 64 dims, out = attn(x) @ Wo^T.

Sharding (8 cores): core c handles batch b = c // 4 and the 4 heads
h in [4*(c%4), 4*(c%4)+4). Each core computes its q/k/v projections
(256 output dims), local attention, and a partial output projection
[4096, 1024] over its 256 contraction dims. The host sums the 4 partials
per batch and adds the (softmax + 1e-9) rank-1 correction plus biases.

V2 dataflow — transposed scores. Per key block jb (128 keys), compute
S^T[j, i] = K_jb^T . Q for the 256 queries {jb, jb+1} directly via
matmul(lhsT=kT-block, rhs=qT-2blocks). exp + band-mask multiply give
P^T[j, i] with keys on partitions — exactly the lhsT layout the PV
matmul needs, so no per-tile PE transposes of P. Each P^T tile is
consumed by the two query blocks jb (diagonal half) and jb+1 (lower
half). A v column of ones emits the softmax denominator for free.

Device layouts per core:
  qT  [dk_on_partitions, 2, 4224]  (one zero query block appended so
                                    every jb computes a uniform N=256)
  kT  [dk_on_partitions, 2, 4096]
  v   [j_on_partitions, 32, 4*66]  (col 64 of each head group is 1.0)
  sT  [j, 4*256] psum; exp (ACT) -> mask mul (DVE) -> pT f16
  ctx [i, 4*65] psum -> normalized f16 -> PE transpose -> out proj
  out f16 partials, upcast + summed on host.
"""

import numpy as np
from contextlib import ExitStack

D_MODEL = 1024
SEQ = 4096
BATCH = 2
D_K = 64
O = 256            # head dims per core (4 heads x 64)
WIN = 128
SCALE = 0.125      # 1/sqrt(64)
N_CORES = 8
NB = SEQ // 128    # 32 query/key blocks
NST = SEQ // 512   # 8 projection column tiles

_CACHE = {}


def _build_program():
    import concourse.tile as tile
    from concourse import bacc, mybir

    f16 = mybir.dt.float16
    f32 = mybir.dt.float32
    AF = mybir.ActivationFunctionType

    nc = bacc.Bacc("TRN2", target_bir_lowering=False, debug=False,
                   num_devices=N_CORES)

    xt_d = nc.dram_tensor("xt", [D_MODEL, SEQ], f16, kind="ExternalInput").ap()
    wq_d = nc.dram_tensor("wq", [D_MODEL, O], f16, kind="ExternalInput").ap()
    wk_d = nc.dram_tensor("wk", [D_MODEL, O], f16, kind="ExternalInput").ap()
    wv_d = nc.dram_tensor("wv", [D_MODEL, O], f16, kind="ExternalInput").ap()
    wo_d = nc.dram_tensor("wo", [O, D_MODEL], f16, kind="ExternalInput").ap()
    mt_d = nc.dram_tensor("maskt", [128, 1024], f16, kind="ExternalInput").ap()
    out_d = nc.dram_tensor("out", [SEQ, D_MODEL], f16, kind="ExternalOutput").ap()

    with tile.TileContext(nc) as tc, ExitStack() as ctx:
        consts = ctx.enter_context(tc.tile_pool(name="consts", bufs=1))
        store = ctx.enter_context(tc.tile_pool(name="store", bufs=1))
        xts = ctx.enter_context(tc.tile_pool(name="xts", bufs=2))
        pms = ctx.enter_context(tc.tile_pool(name="pms", bufs=3))
        pts = ctx.enter_context(tc.tile_pool(name="pts", bufs=6))
        cns = ctx.enter_context(tc.tile_pool(name="cns", bufs=3))
        cts = ctx.enter_context(tc.tile_pool(name="cts", bufs=3))
        recs = ctx.enter_context(tc.tile_pool(name="recs", bufs=4))
        outs = ctx.enter_context(tc.tile_pool(name="outs", bufs=4))
        pp = ctx.enter_context(tc.tile_pool(name="pp", bufs=2, space="PSUM"))
        ppo = ctx.enter_context(tc.tile_pool(name="ppo", bufs=2, space="PSUM"))
        ps4 = ctx.enter_context(tc.tile_pool(name="ps4", bufs=2, space="PSUM"))
        pctx = ctx.enter_context(tc.tile_pool(name="pctx", bufs=1, space="PSUM"))
        ptp = ctx.enter_context(tc.tile_pool(name="ptp", bufs=1, space="PSUM"))

        # ---- constants ----
        wq_sb = consts.tile([128, 8, O], f16)
        wk_sb = consts.tile([128, 8, O], f16)
        wv_sb = consts.tile([128, 8, O], f16)
        nc.sync.dma_start(out=wq_sb, in_=wq_d.rearrange("(a p) o -> p a o", p=128))
        nc.sync.dma_start(out=wk_sb, in_=wk_d.rearrange("(a p) o -> p a o", p=128))
        nc.sync.dma_start(out=wv_sb, in_=wv_d.rearrange("(a p) o -> p a o", p=128))
        wo_sb = consts.tile([128, 2, D_MODEL], f16)
        nc.sync.dma_start(out=wo_sb, in_=wo_d.rearrange("(a p) m -> p a m", p=128))
        mt_sb = consts.tile([128, 1024], f16)
        nc.sync.dma_start(out=mt_sb, in_=mt_d)
        ident = consts.tile([128, 128], f16)
        from concourse.masks import make_identity
        make_identity(nc, ident)

        qT = store.tile([128, 2, SEQ + 128], f16)   # zero query block appended
        kT = store.tile([128, 2, SEQ], f16)
        v = store.tile([128, NB, 4 * (D_K + 2)], f16)
        nc.vector.memset(qT[:, :, SEQ:SEQ + 128], 0.0)
        v4 = v.rearrange("p j (h e) -> p j h e", e=D_K + 2)
        for h in range(4):
            nc.vector.memset(v4[:, :, h, D_K:D_K + 2], 1.0)

        # pm/pT column group per local head h
        colh = lambda h: (h % 2) * 512 + (h // 2) * 256

        # ---- projection chunks: q/k/v for 512 sequence positions ----
        xt_src = xt_d.rearrange("(a p) s -> p a s", p=128)
        xt_tiles = {}

        def xt_load(st):
            s0 = st * 512
            xt = xts.tile([128, 8, 512], f16, tag="xt")
            nc.sync.dma_start(out=xt, in_=xt_src[:, :, s0:s0 + 512])
            xt_tiles[st] = xt

        def qk_chunk(st, w_sb, dst, ot):
            s0 = st * 512
            xt = xt_tiles[st]
            ps = pp.tile([128, 512], mybir.dt.float32, tag="pp")
            for dc in range(8):
                nc.tensor.matmul(
                    ps,
                    lhsT=w_sb[:, dc, ot * 128:(ot + 1) * 128],
                    rhs=xt[:, dc, :],
                    start=(dc == 0), stop=(dc == 7))
            cp = nc.scalar.copy if ot == 0 else nc.vector.tensor_copy
            cp(out=dst[:, ot, s0:s0 + 512], in_=ps)

        def v_chunk(st, ss):
            jb = st * 4 + ss
            xt = xt_tiles[st]
            ps = pp.tile([128, 512], mybir.dt.float32, tag="pp")
            for dc in range(8):
                nc.tensor.matmul(
                    ps[:, 0:O],
                    lhsT=xt[:, dc, ss * 128:(ss + 1) * 128],
                    rhs=wv_sb[:, dc, :],
                    start=(dc == 0), stop=(dc == 7))
            nc.vector.tensor_copy(
                out=v4[:, jb, :, 0:D_K],
                in_=ps[:, 0:O].rearrange("p (h e) -> p h e", e=D_K))

        # ---- transposed score tile for key block jb (queries jb, jb+1) ----
        pt_tiles = {}

        def score_unit(jb):
            j0 = jb * 128
            # bank A: heads 0,2 (row group 0); bank B: heads 1,3 (row group 1)
            banks = [ps4.tile([128, 512], mybir.dt.float32, tag="s4",
                              name=f"s4_{jb}_{i}") for i in range(2)]
            for h in range(4):
                p0 = (h % 2) * 64
                g = h // 2
                nc.tensor.matmul(
                    banks[h % 2][:, g * 256:(g + 1) * 256],
                    lhsT=kT[p0:p0 + 64, g, j0:j0 + 128],
                    rhs=qT[p0:p0 + 64, g, j0:j0 + 256],
                    start=True, stop=True)
            pm = pms.tile([128, 1024], f16, tag="pm")
            pt = pts.tile([128, 1024], f16, tag="pt", name=f"pt_{jb}")
            # mask each half right after its exp so PV of heads 0/2 can start
            # while bank B's exp is still running
            for bk in range(2):
                half = slice(bk * 512, (bk + 1) * 512)
                nc.scalar.activation(out=pm[:, half], in_=banks[bk],
                                     func=AF.Exp)
                nc.vector.tensor_mul(pt[:, half], pm[:, half], mt_sb[:, half])
            pt_tiles[jb] = pt

        # ---- attention output for query block ib ----
        def out_unit(ib):
            i0 = ib * 128
            cps = pctx.tile([128, 4 * (D_K + 1)], mybir.dt.float32, tag="cps")
            for h in range(4):
                alist = [a for a in (0, 1) if ib - 1 + a >= 0]
                for idx, a in enumerate(alist):
                    src = pt_tiles[ib - 1 + a]
                    c0 = colh(h) + (1 - a) * 128
                    nc.tensor.matmul(
                        cps[:, h * 65:h * 65 + 65],
                        lhsT=src[:, c0:c0 + 128],
                        rhs=v4[:, ib - 1 + a, h, 0:D_K + 1],
                        start=(idx == 0), stop=(idx == len(alist) - 1))
            cn = cns.tile([128, 2, 128], f16, tag="cn")
            rec4 = recs.tile([128, 4], mybir.dt.float32, tag="rec")
            cps4 = cps.rearrange("p (h e) -> p h e", e=D_K + 1)
            nc.vector.reciprocal(
                rec4, cps4[:, :, D_K:D_K + 1].rearrange("p h one -> p (h one)"))
            # split normalization across DVE (h 0,1) and ACT (h 2,3) so the
            # psum evacuation doesn't serialize on one engine
            for h in range(4):
                dst = cn[:, h // 2, (h % 2) * 64:(h % 2) * 64 + 64]
                src = cps[:, h * 65:h * 65 + 64]
                if h < 2:
                    nc.vector.tensor_scalar_mul(dst, src, rec4[:, h:h + 1])
                else:
                    nc.scalar.activation(out=dst, in_=src, func=AF.Copy,
                                         scale=rec4[:, h:h + 1])
            ctp_t = ptp.tile([128, 256], f16, tag="ptp")
            for cc in range(2):
                nc.tensor.transpose(
                    ctp_t[:, cc * 128:(cc + 1) * 128], cn[:, cc, :], ident)
            ct = cts.tile([128, 2, 128], f16, tag="ct")
            nc.scalar.copy(out=ct.rearrange("p a i -> p (a i)"), in_=ctp_t)
            ob = outs.tile([128, 1024], f16, tag="ob")
            for mh in range(2):
                po = ppo.tile([128, 512], mybir.dt.float32, tag="po")
                for cc in range(2):
                    nc.tensor.matmul(
                        po,
                        lhsT=ct[:, cc, :],
                        rhs=wo_sb[:, cc, mh * 512:(mh + 1) * 512],
                        start=(cc == 0), stop=(cc == 1))
                cp = nc.scalar.copy if mh == 0 else nc.vector.tensor_copy
                cp(out=ob[:, mh * 512:(mh + 1) * 512], in_=po)
            nc.sync.dma_start(out=out_d[i0:i0 + 128, :], in_=ob)

        def unit(jb):
            if 0 <= jb < NB:
                score_unit(jb)
                out_unit(jb)

        # ---- interleaved emission: projection chunks feed attention units
        # so the PE always has dense matmul work adjacent to each unit ----
        for st in range(NST):
            xt_load(st)
            qk_chunk(st, wq_sb, qT, 0)
            qk_chunk(st, wq_sb, qT, 1)
            unit(4 * st - 1)
            qk_chunk(st, wk_sb, kT, 0)
            qk_chunk(st, wk_sb, kT, 1)
            v_chunk(st, 0)
            unit(4 * st)
            v_chunk(st, 1)
            unit(4 * st + 1)
            v_chunk(st, 2)
            if st < NST - 1:
                v_chunk(st, 3)
                unit(4 * st + 2)
            else:
                # keep the last v chunk as PE filler for the tail units
                unit(4 * st + 2)
                v_chunk(st, 3)
        unit(NB - 1)
    nc.compile()
    return nc


def get_program():
    if "nc" not in _CACHE:
        _CACHE["nc"] = _build_program()
    return _CACHE["nc"]


def _maskT():
    """Transposed band masks, [128 keys, 256 queries] per head group.

    cols 0-127: queries in the same block as the keys (diagonal): jr <= i.
    cols 128-255: queries one block above the keys: jr >= i.
    """
    r = np.arange(128)[:, None]
    c = np.arange(256)[None, :]
    m = np.where(c < 128, r <= c, r >= c - 128).astype(np.float16)
    return np.tile(m, (1, 4))


def make_in_maps(inputs):
    x = np.asarray(inputs["x"], np.float32)
    Wq = np.asarray(inputs["Wq"], np.float32)
    Wk = np.asarray(inputs["Wk"], np.float32)
    Wv = np.asarray(inputs["Wv"], np.float32)
    Wo = np.asarray(inputs["Wo"], np.float32)
    MT = _maskT()
    in_maps = []
    for core in range(N_CORES):
        b, g = core // 4, core % 4
        sl = slice(g * O, (g + 1) * O)
        in_maps.append({
            "xt": np.ascontiguousarray(x[b].T).astype(np.float16),
            "wq": np.ascontiguousarray((Wq[sl] * SCALE).T).astype(np.float16),
            "wk": np.ascontiguousarray(Wk[sl].T).astype(np.float16),
            "wv": np.ascontiguousarray(Wv[sl].T).astype(np.float16),
            "wo": np.ascontiguousarray(Wo[:, sl].T).astype(np.float16),
            "maskt": MT,
        })
    return in_maps


def combine(results, inputs):
    """Sum per-core partials and add host-side corrections."""
    x = np.asarray(inputs["x"], np.float32)
    Wv = np.asarray(inputs["Wv"], np.float32)
    Wo = np.asarray(inputs["Wo"], np.float32)
    bv = np.asarray(inputs["bv"], np.float32)
    bo = np.asarray(inputs["bo"], np.float32)
    out = np.zeros((BATCH, SEQ, D_MODEL), np.float32)
    for core in range(N_CORES):
        out[core // 4] += results[core]["out"].astype(np.float32)
    # reference adds 1e-9 to every attn prob (including masked ones):
    # ctx += 1e-9 * sum_j v[j]  ->  out += 1e-9 * (sum_j v[j]) @ Wo^T
    for b in range(BATCH):
        vs = x[b].sum(axis=0) @ Wv.T + SEQ * bv
        out[b] += (1e-9 * (vs @ Wo.T) + bo)[None, :]
    return out


def run_cores(in_maps, trace=False, **kw):
    from concourse.bass_utils import run_bass_kernel_spmd
    nc = get_program()
    return run_bass_kernel_spmd(nc, in_maps, core_ids=list(range(N_CORES)),
                                trace=trace, **kw)


def kernel(**inputs):
    in_maps = make_in_maps(inputs)
    res = run_cores(in_maps)
    return combine(res.results, inputs)
